# revision 15
# baseline (speedup 1.0000x reference)
"""Trainium2 Bass kernel for a 2-layer LSTM decoder with vocab projection.

Model (per reference):
  embeddings = emb[text]                       # (N, T, H)
  per step t: x_t = [emb_t, v_t] (N, 1024)
    h1,c1 = LSTMCell(x_t, (h1,c1); W_ih1, W_hh1, b_ih1, b_hh1)     # H=512
    h2,c2 = LSTMCell(h1, (h2,c2); W_ih2, W_hh2, b_ih2, b_hh2)     # KS=512
    pred_t = [h2, v_t] @ W_out.T + b_out       # (N, V), V=16000
  out: (N, T, V)

Constants: V=16000, H=VS=KS=512, N=32, T=128.

Sharding: the LSTM recurrence is replicated on all 8 cores (it is
latency-bound, not batch-bound); the output projection is sharded over the
vocab dimension (padded to 16384 = 8 x 2048 rows).

Layouts (device, per core):
  pos = t*32 + b  (time-major positions, 4096 total)
  state/gate partition layout: partition = 32*c + b  (c = hidden chunk 0..3)
  gate free layout: 128*q + u, quarters ordered (i, f, o, g)
  "T" buffers (feature-major): buf[u, c, pos] = x[pos, 128*c + u]

Matmuls are bf16 with fp32 PSUM accumulation; recurrence matmuls use
4x column tiling (each col-group j computes hidden chunk j for all 4 gate
quarters, batch in PE columns).
"""

import numpy as np
import ml_dtypes

V, H, VS, KS = 16000, 512, 512, 512
N, T = 32, 128
NC = 8
VPAD = 16384
VSH = VPAD // NC          # 2048 vocab rows per core
NPOS = N * T              # 4096
BF16 = ml_dtypes.bfloat16

# gate quarter order in the free dim: i, f, o, g
_QMAP = (0, 1, 3, 2)      # free-slot -> original quarter index


def _gate_cols(nH):
    """Column-permutation index [4, 4*128]: [group j, 128*qslot + u] ->
    original gate column 512*q + 128*j + u (for gate dim 4*nH, nH=512)."""
    j = np.arange(4)[:, None, None]
    qs = np.arange(4)[None, :, None]
    u = np.arange(128)[None, None, :]
    q = np.array(_QMAP)[qs]
    cols = nH * q + 128 * j + u
    return cols.reshape(4, 512)


def _prep_host(inputs):
    """Host-side layout prep. Returns (shared_map, per_core_extra)."""
    text = np.asarray(inputs["text"])
    values = np.asarray(inputs["values"], dtype=np.float32)
    emb = np.asarray(inputs["emb"], dtype=np.float32)

    # text: (N, T) -> pos-major flat -> [128, 32] int32 (partition p of block
    # blk holds token for pos = blk*128 + p)
    text_tm = np.ascontiguousarray(text.T).reshape(-1)        # pos = t*32+b
    text_dev = np.ascontiguousarray(
        text_tm.reshape(NPOS // 128, 128).T).astype(np.int32)  # [128, 32]

    # values: (T, N, 512) -> vT[u, c, pos]
    v = values.reshape(NPOS, VS)
    vT = np.ascontiguousarray(
        v.T.reshape(4, 128, NPOS).transpose(1, 0, 2)).astype(BF16)

    emb_bf = emb.astype(BF16)

    cols = _gate_cols(H)  # [4, 512]

    def stack_w(w_list, bias):
        """w_list: list of (rows 2048 x in_features) pieces along K;
        returns [128, nk+1, 4, 512] bf16 with bias in last k-chunk row 0."""
        wfull = np.concatenate(w_list, axis=1)               # [2048, K]
        K = wfull.shape[1]
        nk = K // 128
        wsel = wfull[cols]                                   # [4, 512, K]
        ws = wsel.transpose(2, 0, 1).reshape(nk, 128, 4, 512)
        ws = ws.transpose(1, 0, 2, 3)                        # [128, nk, 4, 512]
        out = np.zeros((128, nk + 1, 4, 512), dtype=np.float32)
        out[:, :nk] = ws
        out[0, nk] = bias[cols]                              # [4, 512]
        return out.astype(BF16)

    W1s = stack_w(
        [np.asarray(inputs["W_ih1"], dtype=np.float32),
         np.asarray(inputs["W_hh1"], dtype=np.float32)],
        np.asarray(inputs["b_ih1"], dtype=np.float32)
        + np.asarray(inputs["b_hh1"], dtype=np.float32))      # [128,13,4,512]
    W2s = stack_w(
        [np.asarray(inputs["W_ih2"], dtype=np.float32),
         np.asarray(inputs["W_hh2"], dtype=np.float32)],
        np.asarray(inputs["b_ih2"], dtype=np.float32)
        + np.asarray(inputs["b_hh2"], dtype=np.float32))      # [128,9,4,512]

    # output projection: pad vocab to 16384, shard 2048 rows per core
    W_out = np.asarray(inputs["W_out"], dtype=np.float32)
    b_out = np.asarray(inputs["b_out"], dtype=np.float32)
    Wp = np.zeros((VPAD, KS + VS), dtype=np.float32)
    Wp[:V] = W_out
    bp = np.zeros(VPAD, dtype=np.float32)
    bp[:V] = b_out

    shared = {"text_dev": text_dev, "vT": vT, "emb": emb_bf,
              "W1s": W1s, "W2s": W2s}
    per_core = []
    for c in range(NC):
        wsh = Wp[c * VSH:(c + 1) * VSH]                       # [2048, 1024]
        WoT = np.ascontiguousarray(
            wsh.T.reshape(8, 128, VSH).transpose(1, 0, 2)).astype(BF16)
        bo = np.ascontiguousarray(
            bp[c * VSH:(c + 1) * VSH].reshape(16, 128).T)     # [128, 16] f32
        per_core.append({"WoT": WoT, "bo": bo})
    return shared, per_core


def _build(t_steps=T, debug=False):
    import concourse.bacc as bacc
    import concourse.bass as bass
    import concourse.mybir as mybir
    import concourse.tile as tile
    from concourse.masks import make_identity

    fp32 = mybir.dt.float32
    bf16 = mybir.dt.bfloat16
    AF = mybir.ActivationFunctionType

    nc = bacc.Bacc("TRN2", target_bir_lowering=False, debug=False,
                   num_devices=NC)

    d_text = nc.declare_dram_parameter("text_dev", [128, 32], mybir.dt.int32,
                                       isOutput=False)
    d_emb = nc.declare_dram_parameter("emb", [V, H], bf16, isOutput=False)
    d_vT = nc.declare_dram_parameter("vT", [128, 4, NPOS], bf16,
                                     isOutput=False)
    d_W1s = nc.declare_dram_parameter("W1s", [128, 13, 4, 512], bf16,
                                      isOutput=False)
    d_W2s = nc.declare_dram_parameter("W2s", [128, 9, 4, 512], bf16,
                                      isOutput=False)
    d_WoT = nc.declare_dram_parameter("WoT", [128, 8, VSH], bf16,
                                      isOutput=False)
    d_bo = nc.declare_dram_parameter("bo", [128, 16], fp32, isOutput=False)
    d_out = nc.declare_dram_parameter("out", [VSH, NPOS], fp32, isOutput=True)
    d_h1dbg = d_h2dbg = None
    if debug:
        d_h1dbg = nc.declare_dram_parameter(
            "h1dbg", [128, t_steps * 128], mybir.dt.bfloat16, isOutput=True)
        d_h2dbg = nc.declare_dram_parameter(
            "h2dbg", [128, t_steps * 128], mybir.dt.bfloat16, isOutput=True)

    n_blocks = (t_steps * 32 + 127) // 128  # 128-pos gather blocks

    with tile.TileContext(nc) as tc:
        with (
            tc.tile_pool(name="persist", bufs=1) as persist,
            tc.tile_pool(name="gather", bufs=3) as gpool,
            tc.tile_pool(name="embT", bufs=8) as epool,
            tc.tile_pool(name="state", bufs=2) as spool,
            tc.tile_pool(name="work", bufs=3) as wpool,
            tc.tile_pool(name="psg", bufs=2, space="PSUM") as psg,
            tc.tile_pool(name="pst", bufs=2, space="PSUM") as pst,
            tc.tile_pool(name="proj_w", bufs=2) as projw,
            tc.tile_pool(name="proj_o", bufs=4) as projo,
            tc.tile_pool(name="psp", bufs=2, space="PSUM") as psp,
        ):
            # ---- static tiles ----
            W1 = persist.tile([128, 13, 4, 512], bf16)
            nc.sync.dma_start(W1[:], d_W1s[:])
            W2 = persist.tile([128, 9, 4, 512], bf16)
            nc.sync.dma_start(W2[:], d_W2s[:])
            vT = persist.tile([128, 4, NPOS], bf16)
            nc.sync.dma_start(vT[:], d_vT[:])
            txt = persist.tile([128, 32], mybir.dt.int32)
            nc.sync.dma_start(txt[:], d_text[:])
            bo = persist.tile([128, 16], fp32)
            nc.sync.dma_start(bo[:], d_bo[:])

            ident = persist.tile([128, 128], bf16)
            make_identity(nc, ident[:])
            ones1 = persist.tile([1, 32], bf16)
            nc.gpsimd.memset(ones1[:], 1.0)

            h2T_buf = persist.tile([128, 4, NPOS], bf16)

            # initial states (zeros)
            h1T_prev = spool.tile([128, 128], bf16, tag="h1T")
            nc.gpsimd.memset(h1T_prev[:], 0.0)
            h2T_init = persist.tile([128, 128], bf16)
            nc.gpsimd.memset(h2T_init[:], 0.0)
            c1_prev = spool.tile([128, 128], fp32, tag="c1")
            nc.gpsimd.memset(c1_prev[:], 0.0)
            c2_prev = spool.tile([128, 128], fp32, tag="c2")
            nc.gpsimd.memset(c2_prev[:], 0.0)

            # ---- embedding gather + transpose (produces embT blocks) ----
            embT_tiles = []

            def gather_block(blk):
                g = gpool.tile([128, H], bf16, tag="embg")
                nc.gpsimd.indirect_dma_start(
                    out=g[:], out_offset=None, in_=d_emb[:],
                    in_offset=bass.IndirectOffsetOnAxis(
                        ap=txt[:, blk:blk + 1], axis=0))
                et = epool.tile([128, 4, 128], bf16, tag="embT")
                for c in range(4):
                    pt = pst.tile([128, 128], bf16, tag="tp")
                    nc.tensor.transpose(pt[:], g[:, 128 * c:128 * (c + 1)],
                                        ident[:])
                    nc.scalar.copy(et[:, c, :], pt[:])
                embT_tiles.append(et)

            for blk in range(min(2, n_blocks)):
                gather_block(blk)

            # ---- recurrence ----
            def xpart(tt):
                """Emit lstm1 x-part matmuls for step tt (emb, v, bias chunks)
                into a fresh g1 psum tile; returns the tile."""
                g1n = psg.tile([128, 512], fp32, tag="g1")
                nblk, nr = tt // 4, tt % 4
                etn = embT_tiles[nblk]
                for k in range(9):
                    if k < 4:
                        lhs = etn[:, k, 32 * nr:32 * (nr + 1)]
                        kw = k
                    elif k < 8:
                        lhs = vT[:, k - 4, 32 * tt:32 * (tt + 1)]
                        kw = k
                    else:
                        lhs = ones1[0:1, :]
                        kw = 12
                    for j in range(4):
                        rhs = (W1[:, kw, j, :] if k < 8
                               else W1[0:1, 12, j, :])
                        nc.tensor.matmul(
                            g1n[32 * j:32 * (j + 1), :], lhs, rhs,
                            start=(k == 0), stop=False,
                            skip_group_check=True, tile_position=(0, 32 * j))
                return g1n

            def eltwise(gps, c_prev, cpool_tag, hpool_tag):
                """LSTM cell eltwise from gates psum [128,512] (i,f,o,g).
                Returns (c_new, h_new[bf16])."""
                sig = wpool.tile([128, 384], fp32, tag="sig" + hpool_tag)
                nc.scalar.activation(sig[:], gps[:, 0:384], AF.Sigmoid)
                tg = wpool.tile([128, 128], fp32, tag="tg" + hpool_tag)
                nc.scalar.activation(tg[:], gps[:, 384:512], AF.Tanh)
                t1 = wpool.tile([128, 128], fp32, tag="t1" + hpool_tag)
                nc.vector.tensor_mul(t1[:], sig[:, 0:128], tg[:])
                t2 = wpool.tile([128, 128], fp32, tag="t2" + hpool_tag)
                nc.vector.tensor_mul(t2[:], sig[:, 128:256], c_prev[:])
                c_new = spool.tile([128, 128], fp32, tag=cpool_tag)
                nc.vector.tensor_add(c_new[:], t1[:], t2[:])
                tc_ = wpool.tile([128, 128], fp32, tag="tc" + hpool_tag)
                nc.scalar.activation(tc_[:], c_new[:], AF.Tanh)
                h = wpool.tile([128, 128], bf16, tag="h" + hpool_tag)
                nc.vector.tensor_mul(h[:], sig[:, 256:384], tc_[:])
                return c_new, h

            # prime: x-part for t=0
            g1_next = xpart(0)

            for t in range(t_steps):
                g1 = g1_next
                # lstm1 h-part: W1 k-chunks 8..11, lhsT = h1T_prev chunks
                for k in range(4):
                    lhs = h1T_prev[:, 32 * k:32 * (k + 1)]
                    for j in range(4):
                        nc.tensor.matmul(
                            g1[32 * j:32 * (j + 1), :], lhs,
                            W1[:, 8 + k, j, :], start=False, stop=False,
                            skip_group_check=True, tile_position=(0, 32 * j))

                # lstm2 h2-part + bias: W2 k-chunks 4..7, 8
                g2 = psg.tile([128, 512], fp32, tag="g2")
                for k in range(4):
                    lhs = (h2T_init[:, 32 * k:32 * (k + 1)] if t == 0
                           else h2T_buf[:, k, 32 * (t - 1):32 * t])
                    for j in range(4):
                        nc.tensor.matmul(
                            g2[32 * j:32 * (j + 1), :], lhs,
                            W2[:, 4 + k, j, :],
                            start=(k == 0), stop=False,
                            skip_group_check=True, tile_position=(0, 32 * j))
                for j in range(4):
                    nc.tensor.matmul(
                        g2[32 * j:32 * (j + 1), :], ones1[0:1, :],
                        W2[0:1, 8, j, :], start=False, stop=False,
                        skip_group_check=True, tile_position=(0, 32 * j))

                # eltwise lstm1
                c1_new, h1 = eltwise(g1, c1_prev, "c1", "1")
                # transpose h1 -> h1T
                pt1 = pst.tile([128, 128], bf16, tag="tp")
                nc.tensor.transpose(pt1[:], h1[:], ident[:])
                h1T = spool.tile([128, 128], bf16, tag="h1T")
                nc.scalar.copy(h1T[:], pt1[:])

                # prefetch gather blocks (stay ~2 blocks ahead)
                want_blk = (t + 4) // 4 + 1
                while len(embT_tiles) <= min(want_blk, n_blocks - 1):
                    gather_block(len(embT_tiles))

                # lstm1 x-part for t+1 (k 0..7 + bias 12)
                if t + 1 < t_steps:
                    g1_next = xpart(t + 1)

                # lstm2 h1-part: W2 k-chunks 0..3
                for k in range(4):
                    lhs = h1T[:, 32 * k:32 * (k + 1)]
                    for j in range(4):
                        nc.tensor.matmul(
                            g2[32 * j:32 * (j + 1), :], lhs,
                            W2[:, k, j, :], start=False,
                            stop=(k == 3 and j == 3), skip_group_check=True,
                            tile_position=(0, 32 * j))

                # eltwise lstm2
                c2_new, h2 = eltwise(g2, c2_prev, "c2", "2")
                if debug:
                    nc.sync.dma_start(
                        d_h1dbg[:, 128 * t:128 * (t + 1)], h1[:])
                    nc.sync.dma_start(
                        d_h2dbg[:, 128 * t:128 * (t + 1)], h2[:])
                pt2 = pst.tile([128, 128], bf16, tag="tp")
                nc.tensor.transpose(pt2[:], h2[:], ident[:])
                nc.scalar.copy(h2T_buf[:, :, 32 * t:32 * (t + 1)],
                               pt2[:].rearrange("p (c b) -> p c b", c=4))

                h1T_prev, c1_prev, c2_prev = h1T, c1_new, c2_new

            # ---- output projection (vocab-major) ----
            n_pt = (t_steps * 32 + 511) // 512
            for vt in range(VSH // 128):
                wo = projw.tile([128, 8, 128], bf16, tag="wo")
                nc.sync.dma_start(wo[:], d_WoT[:, :, 128 * vt:128 * (vt + 1)])
                for pt in range(n_pt):
                    pw = min(512, t_steps * 32 - 512 * pt)
                    ps = psp.tile([128, 512], fp32, tag="pp")
                    for k in range(8):
                        rhs = (h2T_buf[:, k, 512 * pt:512 * pt + pw] if k < 4
                               else vT[:, k - 4, 512 * pt:512 * pt + pw])
                        nc.tensor.matmul(ps[:, :pw], wo[:, k, :], rhs,
                                         start=(k == 0), stop=(k == 7),
                                         skip_group_check=True)
                    ot = projo.tile([128, 512], fp32, tag="ot")
                    if vt % 2 == 0:
                        nc.scalar.activation(ot[:, :pw], ps[:, :pw],
                                             AF.Identity,
                                             bias=bo[:, vt:vt + 1])
                    else:
                        nc.vector.scalar_tensor_tensor(
                            ot[:, :pw], ps[:, :pw], 1.0,
                            bo[:, vt:vt + 1].to_broadcast([128, pw]),
                            op0=mybir.AluOpType.mult,
                            op1=mybir.AluOpType.add)
                    nc.sync.dma_start(
                        d_out[128 * vt:128 * (vt + 1), 512 * pt:512 * pt + pw],
                        ot[:, :pw])

    nc.compile()
    return nc


_CACHE = {}


def _get_nc(t_steps=T):
    if t_steps not in _CACHE:
        _CACHE[t_steps] = _build(t_steps)
    return _CACHE[t_steps]


def kernel(**inputs):
    from concourse.bass_utils import run_bass_kernel_spmd

    shared, per_core = _prep_host(inputs)
    nc = _get_nc(T)
    in_maps = []
    for c in range(NC):
        m = dict(shared)
        m.update(per_core[c])
        in_maps.append(m)
    res = run_bass_kernel_spmd(nc, in_maps, list(range(NC)))
    # gather: each core produced [VSH, NPOS] fp32 (vocab-major)
    cat = np.concatenate([res.results[c]["out"] for c in range(NC)], axis=0)
    cat = cat[:V]                                  # drop padding
    out = cat.reshape(V, T, N).transpose(2, 1, 0)  # (N, T, V)
    return np.ascontiguousarray(out.astype(np.float32))
